# revision 10
# baseline (speedup 1.0000x reference)
"""Diagonal-matrix multiply (column scale) on 8 Trainium2 NeuronCores.

Computes y = x * weight[None, :]  for x:[8192,4096] f32, weight:[4096] f32.
Data-parallel: rows of x sharded 8 ways (1024 rows/core); weight replicated.

Per-core Bass/Tile kernel: stream [128, SUPER*4096] tiles HBM->SBUF,
multiply by a partition-broadcast copy of weight on the vector engine,
stream back. Memory-bound: ~33.5 MB of HBM traffic per core.
"""

import numpy as np

import concourse.bacc as bacc
import concourse.mybir as mybir
from concourse.tile import TileContext
from concourse.bass_utils import run_bass_kernel_spmd

N_CORES = 8
ROWS = 8192
N = 4096
SHARD_ROWS = ROWS // N_CORES  # 1024
P = 128                       # SBUF partitions
SUPER = 2                     # row-blocks fused per tile -> [128, 2, 4096] (4 MiB)
N_TILES = SHARD_ROWS // (P * SUPER)  # 4 super-tiles per core

_nc_cache = {}


def _build(repeat=1):
    """Build (and Bacc-compile) the per-core kernel.

    repeat > 1 wraps the streaming body in a Tile For_i loop that re-runs
    it `repeat` times (idempotent; for wall-clock timing only).
    """
    if repeat in _nc_cache:
        return _nc_cache[repeat]
    nc = bacc.Bacc()
    x = nc.dram_tensor("x", [SHARD_ROWS, N], mybir.dt.float32, kind="ExternalInput")
    w = nc.dram_tensor("weight", [N], mybir.dt.float32, kind="ExternalInput")
    y = nc.dram_tensor("y", [SHARD_ROWS, N], mybir.dt.float32, kind="ExternalOutput")

    # partition p of super-tile i holds rows {(SUPER*i+j)*128 + p : j < SUPER}
    xv = x.rearrange("(n p) m -> p n m", p=P)  # [128, 8, 4096] view
    yv = y.rearrange("(n p) m -> p n m", p=P)

    with TileContext(nc) as tc:
        with (
            tc.tile_pool(name="const", bufs=1) as cpool,
            tc.tile_pool(name="work", bufs=N_TILES) as pool,
        ):
            wtile = cpool.tile([P, N], mybir.dt.float32)
            scratch = cpool.tile([P, 1], mybir.dt.float32)
            # replicate weight into every partition (step-0 partition AP).
            # SWDGE path: keeps all 8 HWDGE sem lanes for the x/y streams
            # (lane reuse would stack a second sync-wait; HW allows one).
            nc.gpsimd.dma_start(out=wtile[:, :], in_=w[None, :].to_broadcast([P, N]))
            # tiny DVE read of wtile: advances DVE's observed tick for the
            # weight DMA sem so the muls below don't each need a second
            # sync-wait (DVE TensorTensor supports only one).
            nc.vector.tensor_copy(out=scratch[:, :], in_=wtile[:, :1])
            wb = wtile[:, None, :].to_broadcast([P, SUPER, N])

            def body():
                for i in range(N_TILES):
                    t = pool.tile([P, SUPER, N], mybir.dt.float32)
                    nc.sync.dma_start(
                        out=t[:, :, :], in_=xv[:, SUPER * i:SUPER * (i + 1), :]
                    )
                    nc.vector.tensor_mul(out=t[:, :, :], in0=t[:, :, :], in1=wb)
                    nc.sync.dma_start(
                        out=yv[:, SUPER * i:SUPER * (i + 1), :], in_=t[:, :, :]
                    )

            if repeat == 1:
                body()
            else:
                with tc.For_i(0, repeat, 1):
                    body()
    nc.compile()
    _nc_cache[repeat] = nc
    return nc


def _shard_inputs(x, weight):
    x = np.ascontiguousarray(np.asarray(x, dtype=np.float32))
    weight = np.ascontiguousarray(np.asarray(weight, dtype=np.float32))
    shards = np.split(x, N_CORES, axis=0)
    return [{"x": s, "weight": weight} for s in shards]


def _run(x, weight, repeat=1, **spmd_kwargs):
    nc = _build(repeat)
    in_maps = _shard_inputs(x, weight)
    res = run_bass_kernel_spmd(nc, in_maps, list(range(N_CORES)), **spmd_kwargs)
    out = np.concatenate([np.asarray(r["y"]) for r in res.results], axis=0)
    return out.astype(np.float32, copy=False), res


def kernel(x, weight):
    out, _ = _run(x, weight)
    return out
